# revision 38
# baseline (speedup 1.0000x reference)
"""Trainium2 Bass kernel for nn_Mlpmoe (moe_routing).

Structure of the problem (B=64, P=256, D=768, H=3072, 6 classes, 5+5 expert atoms):
  - patch tokens [B,256,D] go through a dense MLP (W1 -> gelu -> W2)   (~155 GFLOP)
  - 6 cls tokens  [B,6,D] each go through the TOP-1 of 2 experts
    (atom1 -> gelu -> atom2); the top-1 softmax gate over 2 experts is
    exactly 0/1 after renorm, so only the selected expert is computed.

Sharding over 8 NeuronCores:
  - patch MLP: data-parallel over batch (8 batches/core), moe0 weights replicated,
    computed in bf16 with fp32 PSUM accumulation.
  - cls experts: hidden-dim (H) parallel — core c computes the SELECTED expert for
    all (class,batch) pairs for H-slice [c*384,(c+1)*384) of every atom; per-core
    partial outputs are summed on the host.
  - gates are computed on host from fp32 logits (min observed logit margin
    ~1.7e-3 >> fp32 noise); routing selects which (atom1,atom2) pair runs for
    each (batch,class) column. The emitted program depends only on the 12
    bucket WIDTHS (counts per class/expert), so the NEFF caches per routing
    histogram; batch membership only affects the host gather/scatter.

All activations/weights are fed to the device in bf16 (layouts pre-transposed on
host so no on-device transposes are needed); outputs come back fp32.
"""

import numpy as np
import ml_dtypes

# ---------------------------------------------------------------- constants
NCORES = 8
B, PT, D, H = 64, 256, 768, 3072
NCLS = 6
KD = D // 128            # 6 contraction tiles of 128 over D
MH = H // 128            # 24 tiles over H
HS = H // NCORES         # 384 per-core hidden slice
HK = HS // 128           # 3 tiles over the slice
BPC = B // NCORES        # 8 batches per core
TPC = BPC * PT           # 2048 patch tokens per core
TN = 512                 # token tile (matmul free dim / one PSUM bank)
HTN = TN // 2            # half tile: final group's split width (tail overlap)
NT = TPC // TN           # 4 token tiles
WARM_N = 5               # PE warmup matmuls (head bridge: engine-up -> first
                         # data ~9.9 us with the k-granular two-ring head
                         # split; a short handoff gap only costs its own
                         # length -- whole-run slow clocks are environmental)

PAIRS = [[(0, 3), (3, 0)], [(0, 4), (4, 0)], [(1, 3), (3, 1)],
         [(1, 4), (4, 1)], [(2, 3), (3, 2)], [(2, 4), (4, 2)]]


# ---------------------------------------------------------------- routing
def _gates(x, G_W):
    """Mirror the reference's softmax/top-1/renorm gating in fp32 on host."""
    cls_tokens = np.asarray(x[:, :NCLS], dtype=np.float32)
    logits = np.einsum("bid,ide->bie", cls_tokens, np.asarray(G_W, np.float32))
    m = logits.max(-1, keepdims=True)
    e = np.exp(logits - m)
    gate = e / e.sum(-1, keepdims=True)
    thr = np.sort(gate, axis=-1)[..., -2]
    mask = (gate > thr[..., None]).astype(np.float32)
    g = gate * mask
    g = g / np.clip(g.sum(-1, keepdims=True), 1e-6, None)
    return g  # [B, NCLS, 2], entries exactly 0.0 or 1.0 (or 0/0 on exact ties)


class _Routing:
    """Top-1 routing -> per-atom column buckets + all static offsets.

    Bucket = the batches of one (class, expert) choice; each bucket flows
    through atom1 a (L1 group a) into atom2 c (L2 group c).
    """

    def __init__(self, g):
        sel = np.argmax(g, axis=-1)                    # [B, NCLS]
        tie = g[..., 0] == g[..., 1]                   # both 0.5 or both 0
        buckets = []                                   # (i, a, c, batches)
        for i in range(NCLS):
            for e in range(2):
                a, c = PAIRS[i][e]
                bs = tuple(b for b in range(B)
                           if not tie[b, i] and sel[b, i] == e)
                if bs:
                    buckets.append((i, a, c, bs))
        self.bl1 = [[bk for bk in buckets if bk[1] == j] for j in range(5)]
        self.bl2 = [[bk for bk in buckets if bk[2] == c] for c in range(5)]
        self.n1 = [sum(len(bk[3]) for bk in g1) for g1 in self.bl1]
        self.n2 = [sum(len(bk[3]) for bk in g2) for g2 in self.bl2]
        self.L1OFF = np.cumsum([0] + self.n1).tolist()
        self.COFF = np.cumsum([0] + self.n2).tolist()
        self.NSEL = self.L1OFF[-1]
        # gelu scatter: per L1 group j -> list of (src_off, width, dst c, dst_off)
        self.scat = []
        for j in range(5):
            rows, soff = [], 0
            for (i, a, c, bs) in self.bl1[j]:
                doff = 0
                for bk in self.bl2[c]:
                    if bk[0] == i and bk[1] == a:
                        break
                    doff += len(bk[3])
                rows.append((soff, len(bs), c, doff))
                soff += len(bs)
            self.scat.append(rows)
        # output column order (L2-group major) -> (batch, class, atom2)
        self.colmap = [(b, i, c) for c in range(5)
                       for (i, a, c2, bs) in self.bl2[c] for b in bs]
        # program signature: widths only
        self.sig = tuple((i, a, c, len(bs)) for (i, a, c, bs) in buckets)


_NC_CACHE = {}


def _build_nc(rt):
    """Build + bacc-compile the (SPMD, identical on all cores) Bass program."""
    if rt.sig in _NC_CACHE:
        return _NC_CACHE[rt.sig]

    from contextlib import ExitStack
    import concourse.bass as bass  # noqa: F401
    import concourse.mybir as mybir
    import concourse.tile as tile
    from concourse import bacc
    from concourse.tile import add_dep_helper

    f32 = mybir.dt.float32
    bf16 = mybir.dt.bfloat16
    AF = mybir.ActivationFunctionType
    ts = bass.ts

    NSEL = rt.NSEL
    n1, n2, L1OFF, COFF, scat = rt.n1, rt.n2, rt.L1OFF, rt.COFF, rt.scat

    # disable_frame_to_traceback keeps python source paths out of the BIR, so
    # the compiled-NEFF cache hits no matter which directory kernel.py runs from
    nc = bacc.Bacc("TRN2", target_bir_lowering=False, debug=False,
                   enable_asserts=False, num_devices=NCORES,
                   disable_frame_to_traceback=True)

    # tile-major x layout: [p, t, k, c] so one tile's load is a single
    # 6 KB-per-partition contiguous run (128 descriptors, not 768)
    xp = nc.dram_tensor("xp", [128, NT, KD, TN], bf16, kind="ExternalInput").ap()
    # hm-major weight layout: [p, hm, k, 128] so the first L1 psum group only
    # needs the first small chunk of W1 before matmuls can start
    w1 = nc.dram_tensor("w1", [128, MH, KD, 128], bf16, kind="ExternalInput").ap()
    w2 = nc.dram_tensor("w2", [128, MH, D], bf16, kind="ExternalInput").ap()
    b1 = nc.dram_tensor("b1", [128, MH], f32, kind="ExternalInput").ap()
    b2 = nc.dram_tensor("b2", [128, KD], f32, kind="ExternalInput").ap()
    xc = nc.dram_tensor("xc", [128, KD, NSEL], bf16, kind="ExternalInput").ap()
    a1 = nc.dram_tensor("a1", [5, 128, KD, HS], bf16, kind="ExternalInput").ap()
    a1b = nc.dram_tensor("a1b", [128, 5, HK], f32, kind="ExternalInput").ap()
    a2 = nc.dram_tensor("a2", [5, 128, HK, D], bf16, kind="ExternalInput").ap()
    # patch output in bf16: halves store descriptor-gen + drain (the final
    # two stores sit on the critical tail); costs ~0.2% element rounding,
    # well inside the error budget
    yp = nc.dram_tensor("yp", [128, KD, TPC], bf16, kind="ExternalOutput").ap()
    # cls output: selected columns only, L2-group (atom2) major; each column
    # written once (no on-device accumulate; host sums the 8 H-slice partials)
    yc = nc.dram_tensor("yc", [128, NSEL, KD], f32, kind="ExternalOutput").ap()

    with tile.TileContext(nc) as tc, ExitStack() as ctx:
        wp = ctx.enter_context(tc.tile_pool(name="weights", bufs=1))
        xpool = ctx.enter_context(tc.tile_pool(name="xin", bufs=2))
        hpool = ctx.enter_context(tc.tile_pool(name="hmid", bufs=1))
        opool = ctx.enter_context(tc.tile_pool(name="out", bufs=1))
        cpool = ctx.enter_context(tc.tile_pool(name="cls", bufs=1))
        tpool = ctx.enter_context(tc.tile_pool(name="tmp", bufs=2))
        pspool = ctx.enter_context(tc.tile_pool(name="ps", bufs=8, space="PSUM"))

        # ---- DMA schedule ------------------------------------------------
        # critical path, split across BOTH HWDGE rings: early descriptor
        # service is per-queue-limited (~230 GB/s/queue vs 358 wire), so the
        # scalar ring (idle until the first gelu) carries hm0's weights and
        # x0's back half while the sync ring carries x0's front half. The
        # first real matmul only needs w1-hm0 + x0[k0:3] (~590 KB split
        # 2-ways, ready ~10.5 us instead of ~13 with one queue).
        w1t = wp.tile([128, MH, KD, 128], bf16)
        xt0 = xpool.tile([128, KD, TN], bf16, tag="xt", name="xt0")
        b1t = wp.tile([128, MH], f32)
        b2t = wp.tile([128, KD], f32)
        nc.scalar.dma_start(w1t[:, 0:1], w1[:, 0:1])
        nc.scalar.dma_start(xt0[:, 3:4], xp[:, 0, 3:4])
        nc.scalar.dma_start(xt0[:, 4:6], xp[:, 0, 4:6])
        nc.sync.dma_start(xt0[:, 0:2], xp[:, 0, 0:2])
        nc.sync.dma_start(xt0[:, 2:3], xp[:, 0, 2:3])
        nc.sync.dma_start(b1t[:], b1[:])
        nc.sync.dma_start(w1t[:, 1:2], w1[:, 1:2])
        nc.sync.dma_start(w1t[:, 2:4], w1[:, 2:4])
        nc.sync.dma_start(w1t[:, 4:6], w1[:, 4:6])

        # non-critical loads go on the scalar HWDGE ring, triggered between
        # gelu activations so they don't steal DMA bandwidth from W1/x0
        w2t = wp.tile([128, MH, D], bf16)
        xct = cpool.tile([128, KD, NSEL], bf16)
        a1t = cpool.tile([128, 5, KD, HS], bf16)
        a1bt = cpool.tile([128, 5, HK], f32)
        a2t = cpool.tile([128, 5, HK, D], bf16)
        h2 = [cpool.tile([128, HK, max(n2[c], 1)], bf16, tag=f"h2_{c}",
                         name=f"h2_{c}") for c in range(5)]
        outc = cpool.tile([128, NSEL, KD], f32)

        # second token tile preallocated so its load can be deferred (with
        # bufs=2 the slot is free at kernel start, so an in-loop load would be
        # hoisted into the startup window and steal bandwidth from W1/x0)
        xt1 = xpool.tile([128, KD, TN], bf16, tag="xt", name="xt1")

        # one trigger per gelu slot: remaining W1 chunks stream just ahead of
        # the consuming psum groups, W2 by t0-L2 (~45us), cls inputs by ~220us
        # tiles 2/3 preallocated (pool bufs=2 rotation) with loads deferred to
        # scalar-ring slots so tiles run back-to-back without sync-ring waits
        xt2 = xpool.tile([128, KD, TN], bf16, tag="xt", name="xt2")
        xt3 = xpool.tile([128, KD, TN], bf16, tag="xt", name="xt3")

        # deferred triggers ride the SYNC ring (idle between the head loads
        # and the first stores): their descriptor generation (~0.3-0.6 us
        # DIRECT2D) would otherwise block the scalar ring's gelu chain and
        # stall the PE on psum-bank wraps
        _dl = {}
        _dl[(0, 23)] = lambda: nc.sync.dma_start(xt1[:], xp[:, 1])
        _dl[(0, 13)] = lambda: nc.sync.dma_start(b2t[:], b2[:])
        _dl[(1, 3)] = lambda: nc.sync.dma_start(xt2[:], xp[:, 2])
        _dl[(2, 2)] = lambda: nc.sync.dma_start(xt3[:], xp[:, 3])
        # W1 triggers on consecutive early slots: the sync DMA queue drains
        # in trigger order, so an idle-queue gap early on pushes the whole
        # load train (and the W2 chunks behind it) past their deadlines
        for i, s in enumerate([0, 1, 2, 3, 4, 5, 7, 8, 9]):
            _dl[(0, s)] = lambda i=i: nc.sync.dma_start(
                w1t[:, 6 + 2 * i:8 + 2 * i], w1[:, 6 + 2 * i:8 + 2 * i])
        for i, s in enumerate([6, 10, 11, 12]):
            _dl[(0, s)] = lambda i=i: nc.sync.dma_start(
                w2t[:, 6 * i:6 * (i + 1)], w2[:, 6 * i:6 * (i + 1)])
        _dl[(1, 0)] = lambda: nc.sync.dma_start(xct[:], xc[:])
        for j in range(5):
            _dl[(1, 2 + 2 * j)] = lambda j=j: nc.sync.dma_start(a1t[:, j], a1[j])
            _dl[(1, 14 + 2 * j)] = lambda j=j: nc.sync.dma_start(a2t[:, j], a2[j])
        _dl[(1, 12)] = lambda: nc.sync.dma_start(a1bt[:], a1b[:])

        def _ins(x):
            return getattr(x, "ins", x)

        def deferred_loads(t, hm, act):
            fn = _dl.get((t, hm))
            if fn is not None:
                d = fn()
                # gelu-paced gating: without the edge the scheduler hoists
                # the (dependency-free) trigger to kernel start, where its
                # transfer steals DMA bandwidth from the critical W1/x0 loads
                add_dep_helper(_ins(d), _ins(act),
                               reason="defer bulk load behind gelu")

        # ---- cls expert psum-group emitters ------------------------------
        # Emitted interleaved between tile-3 patch groups: the big 512-wide
        # patch groups give the scalar/vector consumers slack, so the small
        # cls groups never stall the PE on psum-bank reuse.
        def cls_l1_group(j, hm):
            pt = pspool.tile([128, TN], f32, tag="ps", name="pt")
            po = pt[:, :n1[j]]
            for k in range(KD):
                nc.tensor.matmul(po, a1t[:, j, k, ts(hm, 128)],
                                 xct[:, k, L1OFF[j]:L1OFF[j] + n1[j]],
                                 start=(k == 0), stop=(k == KD - 1))
            for (soff, w, c, doff) in scat[j]:
                nc.scalar.activation(h2[c][:, hm, doff:doff + w],
                                     po[:, soff:soff + w], AF.Gelu,
                                     bias=a1bt[:, j, hm, None])

        def cls_l2_group(c, dm):
            pt = pspool.tile([128, TN], f32, tag="ps", name="pt")
            po = pt[:, :n2[c]]
            for hk in range(HK):
                nc.tensor.matmul(po, a2t[:, c, hk, ts(dm, 128)],
                                 h2[c][:, hk, :],
                                 start=(hk == 0), stop=(hk == HK - 1))
            # top-1 gate is exactly 1.0 -> plain copy (no gate multiply)
            nc.vector.tensor_copy(outc[:, COFF[c]:COFF[c] + n2[c], dm], po)

        CLS_L1 = [(j, hm) for j in range(5) if n1[j] for hm in range(HK)]
        CLS_L2 = [(c, dm) for c in range(5) if n2[c] for dm in range(KD)]
        NL2EARLY = sum(KD for c in range(3) if n2[c])   # groups c<3
        nslots = MH - len(CLS_L1)
        per, extra = divmod(len(CLS_L2), nslots)
        CLS_L2_PER_SLOT = [0] * len(CLS_L1) + [
            per + (1 if s < extra else 0) for s in range(nslots)]

        def cls_slot(slot, emitted):
            """Emit this tile-3 L1 slot's share of cls work; return count."""
            if slot < len(CLS_L1):
                cls_l1_group(*CLS_L1[slot])
                return emitted
            for _ in range(CLS_L2_PER_SLOT[slot]):
                cls_l2_group(*CLS_L2[emitted])
                emitted += 1
                if emitted == NL2EARLY:
                    # atom2 groups 0-2 complete: stream their columns out early
                    nc.sync.dma_start(yc[:, :COFF[3]], outc[:, :COFF[3]])
                elif emitted == len(CLS_L2):
                    nc.sync.dma_start(yc[:, COFF[3]:], outc[:, COFF[3]:])
            return emitted

        # ---- patch MLP tile body ----------------------------------------
        def patch_tile(t, xt):
            ht = hpool.tile([128, MH, TN], bf16, tag="ht", name="ht")
            cls_emitted = 0
            for hm in range(MH):
                pt = pspool.tile([128, TN], f32, tag="ps", name="pt")
                for k in range(KD):
                    nc.tensor.matmul(pt[:], w1t[:, hm, k, :], xt[:, k, :],
                                     start=(k == 0), stop=(k == KD - 1))
                act = nc.scalar.activation(ht[:, hm, :], pt[:], AF.Gelu,
                                           bias=b1t[:, hm, None])
                deferred_loads(t, hm, act)
                if t == NT - 1:
                    cls_emitted = cls_slot(hm, cls_emitted)
            ot = opool.tile([128, KD, TN], bf16, tag="ot", name="ot")
            for dm in range(KD):
                if t == NT - 1 and dm == KD - 1:
                    # final group split in two: first half's epilogue+store
                    # overlaps the second half's matmuls. The epilogue adds
                    # run as Copy+bias activations on the SCALAR engine with
                    # the store trigger right behind them on the same ring:
                    # descriptor generation follows the add in queue order,
                    # with no cross-engine semaphore hop before the drain.
                    for hf in range(2):
                        pt = pspool.tile([128, TN], f32, tag="ps", name="pt")
                        po = pt[:, :HTN]
                        hsl = slice(hf * HTN, (hf + 1) * HTN)
                        for k in range(MH):
                            nc.tensor.matmul(po, w2t[:, k, ts(dm, 128)],
                                             ht[:, k, hsl],
                                             start=(k == 0), stop=(k == MH - 1))
                        nc.scalar.activation(ot[:, dm, hsl], po, AF.Identity,
                                             bias=b2t[:, dm, None])
                        dsl = slice(t * TN + hf * HTN, t * TN + (hf + 1) * HTN)
                        if hf == 0:
                            nc.scalar.dma_start(yp[:, dm, dsl], ot[:, dm, hsl])
                        else:
                            # very last store: partition-split across both
                            # (idle) rings -- descriptor gen is count-bound,
                            # so two 64-partition stores gen in parallel
                            nc.scalar.dma_start(yp[0:64, dm, dsl],
                                                ot[0:64, dm, hsl])
                            nc.sync.dma_start(yp[64:128, dm, dsl],
                                              ot[64:128, dm, hsl])
                    continue
                pt = pspool.tile([128, TN], f32, tag="ps", name="pt")
                for k in range(MH):
                    nc.tensor.matmul(pt[:], w2t[:, k, ts(dm, 128)], ht[:, k, :],
                                     start=(k == 0), stop=(k == MH - 1))
                nc.vector.tensor_scalar_add(ot[:, dm, :], pt[:], b2t[:, dm, None])
                # per-dm store so the tail only waits for the last 256 KB
                nc.sync.dma_start(yp[:, dm, ts(t, TN)], ot[:, dm, :])

        # PE warmup: HAM runs the PE at reduced clock until it has been busy
        # ~3.4us. A few dummy matmuls bridge engine-up (~8.4us) to first-data
        # (~10us); the promotion then completes on real rows.
        warm = wp.tile([128, TN], bf16)
        nc.vector.memset(warm[:], 0.0)
        wps = pspool.tile([128, TN], f32, tag="ps", name="warmps")
        for _ in range(WARM_N):
            nc.tensor.matmul(wps[:], warm[:, :128], warm[:], start=True, stop=True)

        # patch tiles back-to-back; cls groups ride inside tile 3's slots
        for t, xt in enumerate([xt0, xt1, xt2, xt3]):
            patch_tile(t, xt)

    nc.compile()
    _NC_CACHE[rt.sig] = nc
    return nc


# ---------------------------------------------------------------- host glue
def _bf(a):
    return np.ascontiguousarray(np.asarray(a), dtype=ml_dtypes.bfloat16)


def _f32(a):
    return np.ascontiguousarray(np.asarray(a), dtype=np.float32)


def _shard_inputs(rt, x, moe0_W1, moe0_b1, moe0_W2, moe0_b2, A1_W, A1_b, A2_W, A2_b):
    x = np.asarray(x, np.float32)

    # shared (replicated) tensors
    # [d, h] -> [p, hm, k, c] with d = k*128+p, h = hm*128+c
    w1v = _bf(np.asarray(moe0_W1, np.float32)).reshape(KD, 128, MH, 128)
    w1v = np.ascontiguousarray(w1v.transpose(1, 2, 0, 3))
    w2v = _bf(np.asarray(moe0_W2, np.float32)).reshape(MH, 128, D).transpose(1, 0, 2)
    w2v = np.ascontiguousarray(w2v)
    b1v = np.ascontiguousarray(_f32(moe0_b1).reshape(MH, 128).T)
    b2v = np.ascontiguousarray(_f32(moe0_b2).reshape(KD, 128).T)

    # stacked selected cls columns (L1 / atom1-group order) -> [128, KD, NSEL]
    xc_f = x[:, :NCLS, :]                                   # [B, 6, D]
    cols = [xc_f[b, i, :] for j in range(5)
            for (i, a, c, bs) in rt.bl1[j] for b in bs]
    stacked = np.stack(cols, axis=0)                        # [NSEL, D]
    xcv = _bf(stacked.T.reshape(KD, 128, rt.NSEL).transpose(1, 0, 2))

    A1_W = np.asarray(A1_W, np.float32)
    A2_W = np.asarray(A2_W, np.float32)
    A1_b = np.asarray(A1_b, np.float32)

    in_maps = []
    for core in range(NCORES):
        hs = slice(core * HS, (core + 1) * HS)
        # per-core patch tokens, tile-major: [128, NT, KD, TN]
        xpc = x[core * BPC:(core + 1) * BPC, NCLS:, :].reshape(TPC, D)
        xpv = _bf(xpc.reshape(NT, TN, KD, 128).transpose(3, 0, 2, 1))
        # atom slices
        a1v = _bf(A1_W[:, :, hs].reshape(5, KD, 128, HS).transpose(0, 2, 1, 3))
        a2v = _bf(A2_W[:, hs, :].reshape(5, HK, 128, D).transpose(0, 2, 1, 3))
        a1bv = np.ascontiguousarray(
            A1_b[:, hs].reshape(5, HK, 128).transpose(2, 0, 1))
        in_maps.append({
            "xp": xpv, "w1": w1v, "w2": w2v, "b1": b1v, "b2": b2v,
            "xc": xcv, "a1": a1v, "a1b": a1bv, "a2": a2v,
        })
    return in_maps


def _combine_outputs(rt, results, A2_b):
    A2_b = np.asarray(A2_b, np.float32)
    out = np.empty((B, NCLS + PT, D), np.float32)
    for core in range(NCORES):
        ypv = np.asarray(results[core]["yp"], np.float32)  # [128, KD, TPC] bf16
        out[core * BPC:(core + 1) * BPC, NCLS:, :] = (
            ypv.transpose(2, 1, 0).reshape(BPC, PT, D))

    ycs = np.zeros((128, rt.NSEL, KD), np.float64)
    for core in range(NCORES):
        ycs += results[core]["yc"].reshape(128, rt.NSEL, KD)
    # [128, col, KD] -> [col, D] with D = kd*128 + p
    cols = ycs.transpose(1, 2, 0).reshape(rt.NSEL, D).astype(np.float32)
    cls_out = np.zeros((B, NCLS, D), np.float32)   # tie rows stay exactly 0
    for col, (b, i, c) in enumerate(rt.colmap):
        cls_out[b, i] = cols[col] + A2_b[c]
    out[:, :NCLS, :] = cls_out
    return out


def _run(inputs, trace=False, trace_kwargs=None):
    from concourse.bass_utils import run_bass_kernel_spmd

    g = _gates(inputs["x"], inputs["G_W"])
    rt = _Routing(g)
    nc = _build_nc(rt)
    in_maps = _shard_inputs(
        rt, inputs["x"], inputs["moe0_W1"], inputs["moe0_b1"], inputs["moe0_W2"],
        inputs["moe0_b2"], inputs["A1_W"], inputs["A1_b"], inputs["A2_W"],
        inputs["A2_b"])
    res = run_bass_kernel_spmd(nc, in_maps, core_ids=list(range(NCORES)),
                               trace=trace, **(trace_kwargs or {}))
    out = _combine_outputs(rt, res.results, inputs["A2_b"])
    return out, res


def kernel(**inputs) -> np.ndarray:
    out, _ = _run(inputs, trace=False)
    return out


# revision 41
# speedup vs baseline: 1.1941x; 1.1941x over previous
"""Trainium2 Bass kernel for nn_Mlpmoe (moe_routing).

Structure of the problem (B=64, P=256, D=768, H=3072, 6 classes, 5+5 expert atoms):
  - patch tokens [B,256,D] go through a dense MLP (W1 -> gelu -> W2)   (~155 GFLOP)
  - 6 cls tokens  [B,6,D] each go through the TOP-1 of 2 experts
    (atom1 -> gelu -> atom2); the top-1 softmax gate over 2 experts is
    exactly 0/1 after renorm, so only the selected expert is computed.

Sharding over 8 NeuronCores:
  - patch MLP: data-parallel over batch (8 batches/core), moe0 weights replicated,
    computed in bf16 with fp32 PSUM accumulation.
  - cls experts: hidden-dim (H) parallel — core c computes the SELECTED expert for
    all (class,batch) pairs for H-slice [c*384,(c+1)*384) of every atom; per-core
    partial outputs are summed on the host.
  - gates are computed on host from fp32 logits (min observed logit margin
    ~1.7e-3 >> fp32 noise); routing selects which (atom1,atom2) pair runs for
    each (batch,class) column. The emitted program depends only on the 12
    bucket WIDTHS (counts per class/expert), so the NEFF caches per routing
    histogram; batch membership only affects the host gather/scatter.

All activations/weights are fed to the device in bf16 (layouts pre-transposed on
host so no on-device transposes are needed); outputs come back fp32.
"""

import numpy as np
import ml_dtypes

# ---------------------------------------------------------------- constants
NCORES = 8
B, PT, D, H = 64, 256, 768, 3072
NCLS = 6
KD = D // 128            # 6 contraction tiles of 128 over D
MH = H // 128            # 24 tiles over H
HS = H // NCORES         # 384 per-core hidden slice
HK = HS // 128           # 3 tiles over the slice
BPC = B // NCORES        # 8 batches per core
TPC = BPC * PT           # 2048 patch tokens per core
TN = 512                 # token tile (matmul free dim / one PSUM bank)
HTN = TN // 2            # half tile: final group's split width (tail overlap)
NT = TPC // TN           # 4 token tiles
WARM_N = 9               # PE warmup matmuls (head bridge: engine-up -> first
                         # data ~11.8 us with the two-ring head split; a short
                         # handoff gap only costs its own length -- whole-run
                         # slow clocks are environmental DVFS, not gap-induced.
                         # Finer k-granular splits measured WORSE: on slow-DMA
                         # runs the just-in-time chunks stall hm0/hm1 repeatedly)

PAIRS = [[(0, 3), (3, 0)], [(0, 4), (4, 0)], [(1, 3), (3, 1)],
         [(1, 4), (4, 1)], [(2, 3), (3, 2)], [(2, 4), (4, 2)]]


# ---------------------------------------------------------------- routing
def _gates(x, G_W):
    """Mirror the reference's softmax/top-1/renorm gating in fp32 on host."""
    cls_tokens = np.asarray(x[:, :NCLS], dtype=np.float32)
    logits = np.einsum("bid,ide->bie", cls_tokens, np.asarray(G_W, np.float32))
    m = logits.max(-1, keepdims=True)
    e = np.exp(logits - m)
    gate = e / e.sum(-1, keepdims=True)
    thr = np.sort(gate, axis=-1)[..., -2]
    mask = (gate > thr[..., None]).astype(np.float32)
    g = gate * mask
    g = g / np.clip(g.sum(-1, keepdims=True), 1e-6, None)
    return g  # [B, NCLS, 2], entries exactly 0.0 or 1.0 (or 0/0 on exact ties)


class _Routing:
    """Top-1 routing -> per-atom column buckets + all static offsets.

    Bucket = the batches of one (class, expert) choice; each bucket flows
    through atom1 a (L1 group a) into atom2 c (L2 group c).
    """

    def __init__(self, g):
        sel = np.argmax(g, axis=-1)                    # [B, NCLS]
        tie = g[..., 0] == g[..., 1]                   # both 0.5 or both 0
        buckets = []                                   # (i, a, c, batches)
        for i in range(NCLS):
            for e in range(2):
                a, c = PAIRS[i][e]
                bs = tuple(b for b in range(B)
                           if not tie[b, i] and sel[b, i] == e)
                if bs:
                    buckets.append((i, a, c, bs))
        self.bl1 = [[bk for bk in buckets if bk[1] == j] for j in range(5)]
        self.bl2 = [[bk for bk in buckets if bk[2] == c] for c in range(5)]
        self.n1 = [sum(len(bk[3]) for bk in g1) for g1 in self.bl1]
        self.n2 = [sum(len(bk[3]) for bk in g2) for g2 in self.bl2]
        self.L1OFF = np.cumsum([0] + self.n1).tolist()
        self.COFF = np.cumsum([0] + self.n2).tolist()
        self.NSEL = self.L1OFF[-1]
        # gelu scatter: per L1 group j -> list of (src_off, width, dst c, dst_off)
        self.scat = []
        for j in range(5):
            rows, soff = [], 0
            for (i, a, c, bs) in self.bl1[j]:
                doff = 0
                for bk in self.bl2[c]:
                    if bk[0] == i and bk[1] == a:
                        break
                    doff += len(bk[3])
                rows.append((soff, len(bs), c, doff))
                soff += len(bs)
            self.scat.append(rows)
        # output column order (L2-group major) -> (batch, class, atom2)
        self.colmap = [(b, i, c) for c in range(5)
                       for (i, a, c2, bs) in self.bl2[c] for b in bs]
        # program signature: widths only
        self.sig = tuple((i, a, c, len(bs)) for (i, a, c, bs) in buckets)


_NC_CACHE = {}


def _build_nc(rt):
    """Build + bacc-compile the (SPMD, identical on all cores) Bass program."""
    if rt.sig in _NC_CACHE:
        return _NC_CACHE[rt.sig]

    from contextlib import ExitStack
    import concourse.bass as bass  # noqa: F401
    import concourse.mybir as mybir
    import concourse.tile as tile
    from concourse import bacc
    from concourse.tile import add_dep_helper

    f32 = mybir.dt.float32
    bf16 = mybir.dt.bfloat16
    AF = mybir.ActivationFunctionType
    ts = bass.ts

    NSEL = rt.NSEL
    n1, n2, L1OFF, COFF, scat = rt.n1, rt.n2, rt.L1OFF, rt.COFF, rt.scat

    # disable_frame_to_traceback keeps python source paths out of the BIR, so
    # the compiled-NEFF cache hits no matter which directory kernel.py runs from
    nc = bacc.Bacc("TRN2", target_bir_lowering=False, debug=False,
                   enable_asserts=False, num_devices=NCORES,
                   disable_frame_to_traceback=True)

    # tile-major x layout: [p, t, k, c] so one tile's load is a single
    # 6 KB-per-partition contiguous run (128 descriptors, not 768)
    xp = nc.dram_tensor("xp", [128, NT, KD, TN], bf16, kind="ExternalInput").ap()
    # hm-major weight layout: [p, hm, k, 128] so the first L1 psum group only
    # needs the first small chunk of W1 before matmuls can start
    w1 = nc.dram_tensor("w1", [128, MH, KD, 128], bf16, kind="ExternalInput").ap()
    w2 = nc.dram_tensor("w2", [128, MH, D], bf16, kind="ExternalInput").ap()
    b1 = nc.dram_tensor("b1", [128, MH], f32, kind="ExternalInput").ap()
    b2 = nc.dram_tensor("b2", [128, KD], f32, kind="ExternalInput").ap()
    xc = nc.dram_tensor("xc", [128, KD, NSEL], bf16, kind="ExternalInput").ap()
    a1 = nc.dram_tensor("a1", [5, 128, KD, HS], bf16, kind="ExternalInput").ap()
    a1b = nc.dram_tensor("a1b", [128, 5, HK], f32, kind="ExternalInput").ap()
    a2 = nc.dram_tensor("a2", [5, 128, HK, D], bf16, kind="ExternalInput").ap()
    # patch output in bf16: halves store descriptor-gen + drain (the final
    # two stores sit on the critical tail); costs ~0.2% element rounding,
    # well inside the error budget
    yp = nc.dram_tensor("yp", [128, KD, TPC], bf16, kind="ExternalOutput").ap()
    # cls output: selected columns only, L2-group (atom2) major; each column
    # written once (no on-device accumulate; host sums the 8 H-slice partials)
    yc = nc.dram_tensor("yc", [128, NSEL, KD], f32, kind="ExternalOutput").ap()

    with tile.TileContext(nc) as tc, ExitStack() as ctx:
        wp = ctx.enter_context(tc.tile_pool(name="weights", bufs=1))
        xpool = ctx.enter_context(tc.tile_pool(name="xin", bufs=2))
        hpool = ctx.enter_context(tc.tile_pool(name="hmid", bufs=1))
        opool = ctx.enter_context(tc.tile_pool(name="out", bufs=1))
        cpool = ctx.enter_context(tc.tile_pool(name="cls", bufs=1))
        tpool = ctx.enter_context(tc.tile_pool(name="tmp", bufs=2))
        pspool = ctx.enter_context(tc.tile_pool(name="ps", bufs=8, space="PSUM"))

        # ---- DMA schedule ------------------------------------------------
        # critical path, split across BOTH HWDGE rings: early descriptor
        # service is per-queue-limited (~230 GB/s/queue vs 358 wire), so the
        # scalar ring (idle until the first gelu) carries hm0's weights and
        # x0's back half while the sync ring carries x0's front half. The
        # first real matmul only needs w1-hm0 + x0[k0:3] (~590 KB split
        # 2-ways, ready ~10.5 us instead of ~13 with one queue).
        w1t = wp.tile([128, MH, KD, 128], bf16)
        xt0 = xpool.tile([128, KD, TN], bf16, tag="xt", name="xt0")
        b1t = wp.tile([128, MH], f32)
        b2t = wp.tile([128, KD], f32)
        nc.scalar.dma_start(w1t[:, 0:1], w1[:, 0:1])
        nc.scalar.dma_start(xt0[:, 3:6], xp[:, 0, 3:6])
        nc.sync.dma_start(xt0[:, 0:3], xp[:, 0, 0:3])
        nc.sync.dma_start(b1t[:], b1[:])
        nc.sync.dma_start(w1t[:, 1:2], w1[:, 1:2])
        nc.sync.dma_start(w1t[:, 2:4], w1[:, 2:4])
        nc.sync.dma_start(w1t[:, 4:6], w1[:, 4:6])

        # non-critical loads go on the scalar HWDGE ring, triggered between
        # gelu activations so they don't steal DMA bandwidth from W1/x0
        w2t = wp.tile([128, MH, D], bf16)
        xct = cpool.tile([128, KD, NSEL], bf16)
        a1t = cpool.tile([128, 5, KD, HS], bf16)
        a1bt = cpool.tile([128, 5, HK], f32)
        a2t = cpool.tile([128, 5, HK, D], bf16)
        h2 = [cpool.tile([128, HK, max(n2[c], 1)], bf16, tag=f"h2_{c}",
                         name=f"h2_{c}") for c in range(5)]
        outc = cpool.tile([128, NSEL, KD], f32)

        # second token tile preallocated so its load can be deferred (with
        # bufs=2 the slot is free at kernel start, so an in-loop load would be
        # hoisted into the startup window and steal bandwidth from W1/x0)
        xt1 = xpool.tile([128, KD, TN], bf16, tag="xt", name="xt1")

        # one trigger per gelu slot: remaining W1 chunks stream just ahead of
        # the consuming psum groups, W2 by t0-L2 (~45us), cls inputs by ~220us
        # tiles 2/3 preallocated (pool bufs=2 rotation) with loads deferred to
        # scalar-ring slots so tiles run back-to-back without sync-ring waits
        xt2 = xpool.tile([128, KD, TN], bf16, tag="xt", name="xt2")
        xt3 = xpool.tile([128, KD, TN], bf16, tag="xt", name="xt3")

        # deferred triggers ride the SYNC ring (idle between the head loads
        # and the first stores): their descriptor generation (~0.3-0.6 us
        # DIRECT2D) would otherwise block the scalar ring's gelu chain and
        # stall the PE on psum-bank wraps
        _dl = {}
        _dl[(0, 23)] = lambda: nc.sync.dma_start(xt1[:], xp[:, 1])
        _dl[(0, 13)] = lambda: nc.sync.dma_start(b2t[:], b2[:])
        _dl[(1, 3)] = lambda: nc.sync.dma_start(xt2[:], xp[:, 2])
        _dl[(2, 2)] = lambda: nc.sync.dma_start(xt3[:], xp[:, 3])
        # W1 triggers on consecutive early slots: the sync DMA queue drains
        # in trigger order, so an idle-queue gap early on pushes the whole
        # load train (and the W2 chunks behind it) past their deadlines
        for i, s in enumerate([0, 1, 2, 3, 4, 5, 7, 8, 9]):
            _dl[(0, s)] = lambda i=i: nc.sync.dma_start(
                w1t[:, 6 + 2 * i:8 + 2 * i], w1[:, 6 + 2 * i:8 + 2 * i])
        for i, s in enumerate([6, 10, 11, 12]):
            _dl[(0, s)] = lambda i=i: nc.sync.dma_start(
                w2t[:, 6 * i:6 * (i + 1)], w2[:, 6 * i:6 * (i + 1)])
        _dl[(1, 0)] = lambda: nc.sync.dma_start(xct[:], xc[:])
        for j in range(5):
            _dl[(1, 2 + 2 * j)] = lambda j=j: nc.sync.dma_start(a1t[:, j], a1[j])
            _dl[(1, 14 + 2 * j)] = lambda j=j: nc.sync.dma_start(a2t[:, j], a2[j])
        _dl[(1, 12)] = lambda: nc.sync.dma_start(a1bt[:], a1b[:])

        def _ins(x):
            return getattr(x, "ins", x)

        def deferred_loads(t, hm, act):
            fn = _dl.get((t, hm))
            if fn is not None:
                d = fn()
                # gelu-paced gating: without the edge the scheduler hoists
                # the (dependency-free) trigger to kernel start, where its
                # transfer steals DMA bandwidth from the critical W1/x0 loads
                add_dep_helper(_ins(d), _ins(act),
                               reason="defer bulk load behind gelu")

        # ---- cls expert psum-group emitters ------------------------------
        # Emitted interleaved between tile-3 patch groups: the big 512-wide
        # patch groups give the scalar/vector consumers slack, so the small
        # cls groups never stall the PE on psum-bank reuse.
        def cls_l1_group(j, hm):
            pt = pspool.tile([128, TN], f32, tag="ps", name="pt")
            po = pt[:, :n1[j]]
            for k in range(KD):
                nc.tensor.matmul(po, a1t[:, j, k, ts(hm, 128)],
                                 xct[:, k, L1OFF[j]:L1OFF[j] + n1[j]],
                                 start=(k == 0), stop=(k == KD - 1))
            for (soff, w, c, doff) in scat[j]:
                nc.scalar.activation(h2[c][:, hm, doff:doff + w],
                                     po[:, soff:soff + w], AF.Gelu,
                                     bias=a1bt[:, j, hm, None])

        def cls_l2_group(c, dm):
            pt = pspool.tile([128, TN], f32, tag="ps", name="pt")
            po = pt[:, :n2[c]]
            for hk in range(HK):
                nc.tensor.matmul(po, a2t[:, c, hk, ts(dm, 128)],
                                 h2[c][:, hk, :],
                                 start=(hk == 0), stop=(hk == HK - 1))
            # top-1 gate is exactly 1.0 -> plain copy (no gate multiply)
            nc.vector.tensor_copy(outc[:, COFF[c]:COFF[c] + n2[c], dm], po)

        CLS_L1 = [(j, hm) for j in range(5) if n1[j] for hm in range(HK)]
        CLS_L2 = [(c, dm) for c in range(5) if n2[c] for dm in range(KD)]
        NL2EARLY = sum(KD for c in range(3) if n2[c])   # groups c<3
        nslots = MH - len(CLS_L1)
        per, extra = divmod(len(CLS_L2), nslots)
        CLS_L2_PER_SLOT = [0] * len(CLS_L1) + [
            per + (1 if s < extra else 0) for s in range(nslots)]

        def cls_slot(slot, emitted):
            """Emit this tile-3 L1 slot's share of cls work; return count."""
            if slot < len(CLS_L1):
                cls_l1_group(*CLS_L1[slot])
                return emitted
            for _ in range(CLS_L2_PER_SLOT[slot]):
                cls_l2_group(*CLS_L2[emitted])
                emitted += 1
                if emitted == NL2EARLY:
                    # atom2 groups 0-2 complete: stream their columns out early
                    nc.sync.dma_start(yc[:, :COFF[3]], outc[:, :COFF[3]])
                elif emitted == len(CLS_L2):
                    nc.sync.dma_start(yc[:, COFF[3]:], outc[:, COFF[3]:])
            return emitted

        # ---- patch MLP tile body ----------------------------------------
        def patch_tile(t, xt):
            ht = hpool.tile([128, MH, TN], bf16, tag="ht", name="ht")
            cls_emitted = 0
            for hm in range(MH):
                pt = pspool.tile([128, TN], f32, tag="ps", name="pt")
                for k in range(KD):
                    nc.tensor.matmul(pt[:], w1t[:, hm, k, :], xt[:, k, :],
                                     start=(k == 0), stop=(k == KD - 1))
                act = nc.scalar.activation(ht[:, hm, :], pt[:], AF.Gelu,
                                           bias=b1t[:, hm, None])
                deferred_loads(t, hm, act)
                if t == NT - 1:
                    cls_emitted = cls_slot(hm, cls_emitted)
            ot = opool.tile([128, KD, TN], bf16, tag="ot", name="ot")
            for dm in range(KD):
                if t == NT - 1 and dm == KD - 1:
                    # final group split in two: first half's epilogue+store
                    # overlaps the second half's matmuls. The epilogue adds
                    # run as Copy+bias activations on the SCALAR engine with
                    # the store trigger right behind them on the same ring:
                    # descriptor generation follows the add in queue order,
                    # with no cross-engine semaphore hop before the drain.
                    for hf in range(2):
                        pt = pspool.tile([128, TN], f32, tag="ps", name="pt")
                        po = pt[:, :HTN]
                        hsl = slice(hf * HTN, (hf + 1) * HTN)
                        for k in range(MH):
                            nc.tensor.matmul(po, w2t[:, k, ts(dm, 128)],
                                             ht[:, k, hsl],
                                             start=(k == 0), stop=(k == MH - 1))
                        dsl = slice(t * TN + hf * HTN, t * TN + (hf + 1) * HTN)
                        if hf == 0:
                            nc.scalar.activation(ot[:, dm, hsl], po, AF.Identity,
                                                 bias=b2t[:, dm, None])
                            nc.scalar.dma_start(yp[:, dm, dsl], ot[:, dm, hsl])
                        else:
                            # very last piece: epilogue halves run on scalar
                            # and vector in parallel (~240 ns each), then the
                            # store partition-splits across both (idle) rings
                            # -- descriptor gen is count-bound, so two
                            # 64-partition stores gen in parallel
                            hq = HTN // 2
                            nc.scalar.activation(
                                ot[:, dm, HTN:HTN + hq], po[:, :hq],
                                AF.Identity, bias=b2t[:, dm, None])
                            nc.vector.tensor_scalar_add(
                                ot[:, dm, HTN + hq:TN], po[:, hq:],
                                b2t[:, dm, None])
                            nc.scalar.dma_start(yp[0:64, dm, dsl],
                                                ot[0:64, dm, hsl])
                            nc.sync.dma_start(yp[64:128, dm, dsl],
                                              ot[64:128, dm, hsl])
                    continue
                pt = pspool.tile([128, TN], f32, tag="ps", name="pt")
                for k in range(MH):
                    nc.tensor.matmul(pt[:], w2t[:, k, ts(dm, 128)], ht[:, k, :],
                                     start=(k == 0), stop=(k == MH - 1))
                nc.vector.tensor_scalar_add(ot[:, dm, :], pt[:], b2t[:, dm, None])
                # per-dm store so the tail only waits for the last 256 KB
                nc.sync.dma_start(yp[:, dm, ts(t, TN)], ot[:, dm, :])

        # PE warmup: HAM runs the PE at reduced clock until it has been busy
        # ~3.4us. A few dummy matmuls bridge engine-up (~8.4us) to first-data
        # (~10us); the promotion then completes on real rows.
        warm = wp.tile([128, TN], bf16)
        nc.vector.memset(warm[:], 0.0)
        wps = pspool.tile([128, TN], f32, tag="ps", name="warmps")
        for _ in range(WARM_N):
            nc.tensor.matmul(wps[:], warm[:, :128], warm[:], start=True, stop=True)

        # patch tiles back-to-back; cls groups ride inside tile 3's slots
        for t, xt in enumerate([xt0, xt1, xt2, xt3]):
            patch_tile(t, xt)

    nc.compile()
    _NC_CACHE[rt.sig] = nc
    return nc


# ---------------------------------------------------------------- host glue
def _bf(a):
    return np.ascontiguousarray(np.asarray(a), dtype=ml_dtypes.bfloat16)


def _f32(a):
    return np.ascontiguousarray(np.asarray(a), dtype=np.float32)


def _shard_inputs(rt, x, moe0_W1, moe0_b1, moe0_W2, moe0_b2, A1_W, A1_b, A2_W, A2_b):
    x = np.asarray(x, np.float32)

    # shared (replicated) tensors
    # [d, h] -> [p, hm, k, c] with d = k*128+p, h = hm*128+c
    w1v = _bf(np.asarray(moe0_W1, np.float32)).reshape(KD, 128, MH, 128)
    w1v = np.ascontiguousarray(w1v.transpose(1, 2, 0, 3))
    w2v = _bf(np.asarray(moe0_W2, np.float32)).reshape(MH, 128, D).transpose(1, 0, 2)
    w2v = np.ascontiguousarray(w2v)
    b1v = np.ascontiguousarray(_f32(moe0_b1).reshape(MH, 128).T)
    b2v = np.ascontiguousarray(_f32(moe0_b2).reshape(KD, 128).T)

    # stacked selected cls columns (L1 / atom1-group order) -> [128, KD, NSEL]
    xc_f = x[:, :NCLS, :]                                   # [B, 6, D]
    cols = [xc_f[b, i, :] for j in range(5)
            for (i, a, c, bs) in rt.bl1[j] for b in bs]
    stacked = np.stack(cols, axis=0)                        # [NSEL, D]
    xcv = _bf(stacked.T.reshape(KD, 128, rt.NSEL).transpose(1, 0, 2))

    A1_W = np.asarray(A1_W, np.float32)
    A2_W = np.asarray(A2_W, np.float32)
    A1_b = np.asarray(A1_b, np.float32)

    in_maps = []
    for core in range(NCORES):
        hs = slice(core * HS, (core + 1) * HS)
        # per-core patch tokens, tile-major: [128, NT, KD, TN]
        xpc = x[core * BPC:(core + 1) * BPC, NCLS:, :].reshape(TPC, D)
        xpv = _bf(xpc.reshape(NT, TN, KD, 128).transpose(3, 0, 2, 1))
        # atom slices
        a1v = _bf(A1_W[:, :, hs].reshape(5, KD, 128, HS).transpose(0, 2, 1, 3))
        a2v = _bf(A2_W[:, hs, :].reshape(5, HK, 128, D).transpose(0, 2, 1, 3))
        a1bv = np.ascontiguousarray(
            A1_b[:, hs].reshape(5, HK, 128).transpose(2, 0, 1))
        in_maps.append({
            "xp": xpv, "w1": w1v, "w2": w2v, "b1": b1v, "b2": b2v,
            "xc": xcv, "a1": a1v, "a1b": a1bv, "a2": a2v,
        })
    return in_maps


def _combine_outputs(rt, results, A2_b):
    A2_b = np.asarray(A2_b, np.float32)
    out = np.empty((B, NCLS + PT, D), np.float32)
    for core in range(NCORES):
        ypv = np.asarray(results[core]["yp"], np.float32)  # [128, KD, TPC] bf16
        out[core * BPC:(core + 1) * BPC, NCLS:, :] = (
            ypv.transpose(2, 1, 0).reshape(BPC, PT, D))

    ycs = np.zeros((128, rt.NSEL, KD), np.float64)
    for core in range(NCORES):
        ycs += results[core]["yc"].reshape(128, rt.NSEL, KD)
    # [128, col, KD] -> [col, D] with D = kd*128 + p
    cols = ycs.transpose(1, 2, 0).reshape(rt.NSEL, D).astype(np.float32)
    cls_out = np.zeros((B, NCLS, D), np.float32)   # tie rows stay exactly 0
    for col, (b, i, c) in enumerate(rt.colmap):
        cls_out[b, i] = cols[col] + A2_b[c]
    out[:, :NCLS, :] = cls_out
    return out


def _run(inputs, trace=False, trace_kwargs=None):
    from concourse.bass_utils import run_bass_kernel_spmd

    g = _gates(inputs["x"], inputs["G_W"])
    rt = _Routing(g)
    nc = _build_nc(rt)
    in_maps = _shard_inputs(
        rt, inputs["x"], inputs["moe0_W1"], inputs["moe0_b1"], inputs["moe0_W2"],
        inputs["moe0_b2"], inputs["A1_W"], inputs["A1_b"], inputs["A2_W"],
        inputs["A2_b"])
    res = run_bass_kernel_spmd(nc, in_maps, core_ids=list(range(NCORES)),
                               trace=trace, **(trace_kwargs or {}))
    out = _combine_outputs(rt, res.results, inputs["A2_b"])
    return out, res


def kernel(**inputs) -> np.ndarray:
    out, _ = _run(inputs, trace=False)
    return out


# revision 42
# speedup vs baseline: 1.2037x; 1.0081x over previous
"""Trainium2 Bass kernel for nn_Mlpmoe (moe_routing).

Structure of the problem (B=64, P=256, D=768, H=3072, 6 classes, 5+5 expert atoms):
  - patch tokens [B,256,D] go through a dense MLP (W1 -> gelu -> W2)   (~155 GFLOP)
  - 6 cls tokens  [B,6,D] each go through the TOP-1 of 2 experts
    (atom1 -> gelu -> atom2); the top-1 softmax gate over 2 experts is
    exactly 0/1 after renorm, so only the selected expert is computed.

Sharding over 8 NeuronCores:
  - patch MLP: data-parallel over batch (8 batches/core), moe0 weights replicated,
    computed in bf16 with fp32 PSUM accumulation.
  - cls experts: hidden-dim (H) parallel — core c computes the SELECTED expert for
    all (class,batch) pairs for H-slice [c*384,(c+1)*384) of every atom; per-core
    partial outputs are summed on the host.
  - gates are computed on host from fp32 logits (min observed logit margin
    ~1.7e-3 >> fp32 noise); routing selects which (atom1,atom2) pair runs for
    each (batch,class) column. The emitted program depends only on the 12
    bucket WIDTHS (counts per class/expert), so the NEFF caches per routing
    histogram; batch membership only affects the host gather/scatter.

All activations/weights are fed to the device in bf16 (layouts pre-transposed on
host so no on-device transposes are needed); outputs come back fp32.
"""

import numpy as np
import ml_dtypes

# ---------------------------------------------------------------- constants
NCORES = 8
B, PT, D, H = 64, 256, 768, 3072
NCLS = 6
KD = D // 128            # 6 contraction tiles of 128 over D
MH = H // 128            # 24 tiles over H
HS = H // NCORES         # 384 per-core hidden slice
HK = HS // 128           # 3 tiles over the slice
BPC = B // NCORES        # 8 batches per core
TPC = BPC * PT           # 2048 patch tokens per core
TN = 512                 # token tile (matmul free dim / one PSUM bank)
HTN = TN // 2            # half tile: final group's split width (tail overlap)
NT = TPC // TN           # 4 token tiles
WARM_N = 9               # PE warmup matmuls (head bridge: engine-up -> first
                         # data ~11.8 us with the two-ring head split; a short
                         # handoff gap only costs its own length -- whole-run
                         # slow clocks are environmental DVFS, not gap-induced.
                         # Finer k-granular splits measured WORSE: on slow-DMA
                         # runs the just-in-time chunks stall hm0/hm1 repeatedly)

PAIRS = [[(0, 3), (3, 0)], [(0, 4), (4, 0)], [(1, 3), (3, 1)],
         [(1, 4), (4, 1)], [(2, 3), (3, 2)], [(2, 4), (4, 2)]]


# ---------------------------------------------------------------- routing
def _gates(x, G_W):
    """Mirror the reference's softmax/top-1/renorm gating in fp32 on host."""
    cls_tokens = np.asarray(x[:, :NCLS], dtype=np.float32)
    logits = np.einsum("bid,ide->bie", cls_tokens, np.asarray(G_W, np.float32))
    m = logits.max(-1, keepdims=True)
    e = np.exp(logits - m)
    gate = e / e.sum(-1, keepdims=True)
    thr = np.sort(gate, axis=-1)[..., -2]
    mask = (gate > thr[..., None]).astype(np.float32)
    g = gate * mask
    g = g / np.clip(g.sum(-1, keepdims=True), 1e-6, None)
    return g  # [B, NCLS, 2], entries exactly 0.0 or 1.0 (or 0/0 on exact ties)


class _Routing:
    """Top-1 routing -> per-atom column buckets + all static offsets.

    Bucket = the batches of one (class, expert) choice; each bucket flows
    through atom1 a (L1 group a) into atom2 c (L2 group c).
    """

    def __init__(self, g):
        sel = np.argmax(g, axis=-1)                    # [B, NCLS]
        tie = g[..., 0] == g[..., 1]                   # both 0.5 or both 0
        buckets = []                                   # (i, a, c, batches)
        for i in range(NCLS):
            for e in range(2):
                a, c = PAIRS[i][e]
                bs = tuple(b for b in range(B)
                           if not tie[b, i] and sel[b, i] == e)
                if bs:
                    buckets.append((i, a, c, bs))
        self.bl1 = [[bk for bk in buckets if bk[1] == j] for j in range(5)]
        self.bl2 = [[bk for bk in buckets if bk[2] == c] for c in range(5)]
        self.n1 = [sum(len(bk[3]) for bk in g1) for g1 in self.bl1]
        self.n2 = [sum(len(bk[3]) for bk in g2) for g2 in self.bl2]
        self.L1OFF = np.cumsum([0] + self.n1).tolist()
        self.COFF = np.cumsum([0] + self.n2).tolist()
        self.NSEL = self.L1OFF[-1]
        # gelu scatter: per L1 group j -> list of (src_off, width, dst c, dst_off)
        self.scat = []
        for j in range(5):
            rows, soff = [], 0
            for (i, a, c, bs) in self.bl1[j]:
                doff = 0
                for bk in self.bl2[c]:
                    if bk[0] == i and bk[1] == a:
                        break
                    doff += len(bk[3])
                rows.append((soff, len(bs), c, doff))
                soff += len(bs)
            self.scat.append(rows)
        # output column order (L2-group major) -> (batch, class, atom2)
        self.colmap = [(b, i, c) for c in range(5)
                       for (i, a, c2, bs) in self.bl2[c] for b in bs]
        # program signature: widths only
        self.sig = tuple((i, a, c, len(bs)) for (i, a, c, bs) in buckets)


_NC_CACHE = {}


def _build_nc(rt):
    """Build + bacc-compile the (SPMD, identical on all cores) Bass program."""
    if rt.sig in _NC_CACHE:
        return _NC_CACHE[rt.sig]

    from contextlib import ExitStack
    import concourse.bass as bass  # noqa: F401
    import concourse.mybir as mybir
    import concourse.tile as tile
    from concourse import bacc
    from concourse.tile import add_dep_helper

    f32 = mybir.dt.float32
    bf16 = mybir.dt.bfloat16
    AF = mybir.ActivationFunctionType
    ts = bass.ts

    NSEL = rt.NSEL
    n1, n2, L1OFF, COFF, scat = rt.n1, rt.n2, rt.L1OFF, rt.COFF, rt.scat

    # disable_frame_to_traceback keeps python source paths out of the BIR, so
    # the compiled-NEFF cache hits no matter which directory kernel.py runs from
    nc = bacc.Bacc("TRN2", target_bir_lowering=False, debug=False,
                   enable_asserts=False, num_devices=NCORES,
                   disable_frame_to_traceback=True)

    # tile-major x layout: [p, t, k, c] so one tile's load is a single
    # 6 KB-per-partition contiguous run (128 descriptors, not 768)
    xp = nc.dram_tensor("xp", [128, NT, KD, TN], bf16, kind="ExternalInput").ap()
    # hm-major weight layout: [p, hm, k, 128] so the first L1 psum group only
    # needs the first small chunk of W1 before matmuls can start
    w1 = nc.dram_tensor("w1", [128, MH, KD, 128], bf16, kind="ExternalInput").ap()
    w2 = nc.dram_tensor("w2", [128, MH, D], bf16, kind="ExternalInput").ap()
    b1 = nc.dram_tensor("b1", [128, MH], f32, kind="ExternalInput").ap()
    b2 = nc.dram_tensor("b2", [128, KD], f32, kind="ExternalInput").ap()
    xc = nc.dram_tensor("xc", [128, KD, NSEL], bf16, kind="ExternalInput").ap()
    a1 = nc.dram_tensor("a1", [5, 128, KD, HS], bf16, kind="ExternalInput").ap()
    a1b = nc.dram_tensor("a1b", [128, 5, HK], f32, kind="ExternalInput").ap()
    a2 = nc.dram_tensor("a2", [5, 128, HK, D], bf16, kind="ExternalInput").ap()
    # patch output in bf16: halves store descriptor-gen + drain (the final
    # two stores sit on the critical tail); costs ~0.2% element rounding,
    # well inside the error budget
    yp = nc.dram_tensor("yp", [128, KD, TPC], bf16, kind="ExternalOutput").ap()
    # cls output: selected columns only, L2-group (atom2) major; each column
    # written once (no on-device accumulate; host sums the 8 H-slice partials)
    yc = nc.dram_tensor("yc", [128, NSEL, KD], f32, kind="ExternalOutput").ap()

    with tile.TileContext(nc) as tc, ExitStack() as ctx:
        wp = ctx.enter_context(tc.tile_pool(name="weights", bufs=1))
        xpool = ctx.enter_context(tc.tile_pool(name="xin", bufs=2))
        hpool = ctx.enter_context(tc.tile_pool(name="hmid", bufs=1))
        opool = ctx.enter_context(tc.tile_pool(name="out", bufs=1))
        cpool = ctx.enter_context(tc.tile_pool(name="cls", bufs=1))
        tpool = ctx.enter_context(tc.tile_pool(name="tmp", bufs=2))
        pspool = ctx.enter_context(tc.tile_pool(name="ps", bufs=8, space="PSUM"))

        # ---- DMA schedule ------------------------------------------------
        # critical path, split across BOTH HWDGE rings: early descriptor
        # service is per-queue-limited (~230 GB/s/queue vs 358 wire), so the
        # scalar ring (idle until the first gelu) carries hm0's weights and
        # x0's back half while the sync ring carries x0's front half. The
        # first real matmul only needs w1-hm0 + x0[k0:3] (~590 KB split
        # 2-ways, ready ~10.5 us instead of ~13 with one queue).
        w1t = wp.tile([128, MH, KD, 128], bf16)
        xt0 = xpool.tile([128, KD, TN], bf16, tag="xt", name="xt0")
        b1t = wp.tile([128, MH], f32)
        b2t = wp.tile([128, KD], f32)
        nc.scalar.dma_start(w1t[:, 0:1], w1[:, 0:1])
        nc.scalar.dma_start(xt0[:, 3:6], xp[:, 0, 3:6])
        nc.sync.dma_start(xt0[:, 0:3], xp[:, 0, 0:3])
        nc.sync.dma_start(b1t[:], b1[:])
        nc.sync.dma_start(w1t[:, 1:2], w1[:, 1:2])
        nc.sync.dma_start(w1t[:, 2:4], w1[:, 2:4])
        nc.sync.dma_start(w1t[:, 4:6], w1[:, 4:6])

        # non-critical loads go on the scalar HWDGE ring, triggered between
        # gelu activations so they don't steal DMA bandwidth from W1/x0
        w2t = wp.tile([128, MH, D], bf16)
        xct = cpool.tile([128, KD, NSEL], bf16)
        a1t = cpool.tile([128, 5, KD, HS], bf16)
        a1bt = cpool.tile([128, 5, HK], f32)
        a2t = cpool.tile([128, 5, HK, D], bf16)
        h2 = [cpool.tile([128, HK, max(n2[c], 1)], bf16, tag=f"h2_{c}",
                         name=f"h2_{c}") for c in range(5)]
        outc = cpool.tile([128, NSEL, KD], f32)

        # second token tile preallocated so its load can be deferred (with
        # bufs=2 the slot is free at kernel start, so an in-loop load would be
        # hoisted into the startup window and steal bandwidth from W1/x0)
        xt1 = xpool.tile([128, KD, TN], bf16, tag="xt", name="xt1")

        # one trigger per gelu slot: remaining W1 chunks stream just ahead of
        # the consuming psum groups, W2 by t0-L2 (~45us), cls inputs by ~220us
        # tiles 2/3 preallocated (pool bufs=2 rotation) with loads deferred to
        # scalar-ring slots so tiles run back-to-back without sync-ring waits
        xt2 = xpool.tile([128, KD, TN], bf16, tag="xt", name="xt2")
        xt3 = xpool.tile([128, KD, TN], bf16, tag="xt", name="xt3")

        # deferred triggers ride the SYNC ring (idle between the head loads
        # and the first stores): their descriptor generation (~0.3-0.6 us
        # DIRECT2D) would otherwise block the scalar ring's gelu chain and
        # stall the PE on psum-bank wraps
        _dl = {}
        _dl[(0, 23)] = lambda: nc.sync.dma_start(xt1[:], xp[:, 1])
        _dl[(0, 13)] = lambda: nc.sync.dma_start(b2t[:], b2[:])
        _dl[(1, 3)] = lambda: nc.sync.dma_start(xt2[:], xp[:, 2])
        _dl[(2, 2)] = lambda: nc.sync.dma_start(xt3[:], xp[:, 3])
        # W1 triggers on consecutive early slots: the sync DMA queue drains
        # in trigger order, so an idle-queue gap early on pushes the whole
        # load train (and the W2 chunks behind it) past their deadlines
        for i, s in enumerate([0, 1, 2, 3, 4, 5, 7, 8, 9]):
            _dl[(0, s)] = lambda i=i: nc.sync.dma_start(
                w1t[:, 6 + 2 * i:8 + 2 * i], w1[:, 6 + 2 * i:8 + 2 * i])
        for i, s in enumerate([6, 10, 11, 12]):
            _dl[(0, s)] = lambda i=i: nc.sync.dma_start(
                w2t[:, 6 * i:6 * (i + 1)], w2[:, 6 * i:6 * (i + 1)])
        _dl[(1, 0)] = lambda: nc.sync.dma_start(xct[:], xc[:])
        for j in range(5):
            _dl[(1, 2 + 2 * j)] = lambda j=j: nc.sync.dma_start(a1t[:, j], a1[j])
            _dl[(1, 14 + 2 * j)] = lambda j=j: nc.sync.dma_start(a2t[:, j], a2[j])
        _dl[(1, 12)] = lambda: nc.sync.dma_start(a1bt[:], a1b[:])

        def _ins(x):
            return getattr(x, "ins", x)

        def deferred_loads(t, hm, act):
            fn = _dl.get((t, hm))
            if fn is not None:
                d = fn()
                # gelu-paced gating: without the edge the scheduler hoists
                # the (dependency-free) trigger to kernel start, where its
                # transfer steals DMA bandwidth from the critical W1/x0 loads
                add_dep_helper(_ins(d), _ins(act),
                               reason="defer bulk load behind gelu")

        # ---- cls expert psum-group emitters ------------------------------
        # Emitted interleaved between tile-3 patch groups: the big 512-wide
        # patch groups give the scalar/vector consumers slack, so the small
        # cls groups never stall the PE on psum-bank reuse.
        def cls_l1_group(j, hm):
            pt = pspool.tile([128, TN], f32, tag="ps", name="pt")
            po = pt[:, :n1[j]]
            for k in range(KD):
                nc.tensor.matmul(po, a1t[:, j, k, ts(hm, 128)],
                                 xct[:, k, L1OFF[j]:L1OFF[j] + n1[j]],
                                 start=(k == 0), stop=(k == KD - 1))
            for (soff, w, c, doff) in scat[j]:
                nc.scalar.activation(h2[c][:, hm, doff:doff + w],
                                     po[:, soff:soff + w], AF.Gelu,
                                     bias=a1bt[:, j, hm, None])

        def cls_l2_group(c, dm):
            pt = pspool.tile([128, TN], f32, tag="ps", name="pt")
            po = pt[:, :n2[c]]
            for hk in range(HK):
                nc.tensor.matmul(po, a2t[:, c, hk, ts(dm, 128)],
                                 h2[c][:, hk, :],
                                 start=(hk == 0), stop=(hk == HK - 1))
            # top-1 gate is exactly 1.0 -> plain copy (no gate multiply)
            nc.vector.tensor_copy(outc[:, COFF[c]:COFF[c] + n2[c], dm], po)

        CLS_L1 = [(j, hm) for j in range(5) if n1[j] for hm in range(HK)]
        CLS_L2 = [(c, dm) for c in range(5) if n2[c] for dm in range(KD)]
        NL2EARLY = sum(KD for c in range(3) if n2[c])   # groups c<3
        nslots = MH - len(CLS_L1)
        per, extra = divmod(len(CLS_L2), nslots)
        CLS_L2_PER_SLOT = [0] * len(CLS_L1) + [
            per + (1 if s < extra else 0) for s in range(nslots)]

        def cls_slot(slot, emitted):
            """Emit this tile-3 L1 slot's share of cls work; return count."""
            if slot < len(CLS_L1):
                cls_l1_group(*CLS_L1[slot])
                return emitted
            for _ in range(CLS_L2_PER_SLOT[slot]):
                cls_l2_group(*CLS_L2[emitted])
                emitted += 1
                if emitted == NL2EARLY:
                    # atom2 groups 0-2 complete: stream their columns out early
                    nc.sync.dma_start(yc[:, :COFF[3]], outc[:, :COFF[3]])
                elif emitted == len(CLS_L2):
                    nc.sync.dma_start(yc[:, COFF[3]:], outc[:, COFF[3]:])
            return emitted

        # ---- patch MLP tile body ----------------------------------------
        def patch_tile(t, xt):
            ht = hpool.tile([128, MH, TN], bf16, tag="ht", name="ht")
            cls_emitted = 0
            for hm in range(MH):
                pt = pspool.tile([128, TN], f32, tag="ps", name="pt")
                for k in range(KD):
                    nc.tensor.matmul(pt[:], w1t[:, hm, k, :], xt[:, k, :],
                                     start=(k == 0), stop=(k == KD - 1))
                act = nc.scalar.activation(ht[:, hm, :], pt[:], AF.Gelu,
                                           bias=b1t[:, hm, None])
                deferred_loads(t, hm, act)
                if t == NT - 1:
                    cls_emitted = cls_slot(hm, cls_emitted)
            ot = opool.tile([128, KD, TN], bf16, tag="ot", name="ot")
            for dm in range(KD):
                if t == NT - 1 and dm == KD - 1:
                    # final group split in two: first half's epilogue+store
                    # overlaps the second half's matmuls. The epilogue adds
                    # run as Copy+bias activations on the SCALAR engine with
                    # the store trigger right behind them on the same ring:
                    # descriptor generation follows the add in queue order,
                    # with no cross-engine semaphore hop before the drain.
                    for hf in range(2):
                        pt = pspool.tile([128, TN], f32, tag="ps", name="pt")
                        po = pt[:, :HTN]
                        hsl = slice(hf * HTN, (hf + 1) * HTN)
                        for k in range(MH):
                            nc.tensor.matmul(po, w2t[:, k, ts(dm, 128)],
                                             ht[:, k, hsl],
                                             start=(k == 0), stop=(k == MH - 1))
                        nc.scalar.activation(ot[:, dm, hsl], po, AF.Identity,
                                             bias=b2t[:, dm, None])
                        dsl = slice(t * TN + hf * HTN, t * TN + (hf + 1) * HTN)
                        if hf == 0:
                            nc.scalar.dma_start(yp[:, dm, dsl], ot[:, dm, hsl])
                        else:
                            # very last store: partition-split across both
                            # (idle) rings for parallel descriptor gen. A
                            # scalar+vector parallel-epilogue variant measured
                            # WORSE (cross-engine sem lag delayed the stores).
                            nc.scalar.dma_start(yp[0:64, dm, dsl],
                                                ot[0:64, dm, hsl])
                            nc.sync.dma_start(yp[64:128, dm, dsl],
                                              ot[64:128, dm, hsl])
                    continue
                pt = pspool.tile([128, TN], f32, tag="ps", name="pt")
                for k in range(MH):
                    nc.tensor.matmul(pt[:], w2t[:, k, ts(dm, 128)], ht[:, k, :],
                                     start=(k == 0), stop=(k == MH - 1))
                nc.vector.tensor_scalar_add(ot[:, dm, :], pt[:], b2t[:, dm, None])
                # per-dm store so the tail only waits for the last 256 KB
                nc.sync.dma_start(yp[:, dm, ts(t, TN)], ot[:, dm, :])

        # PE warmup: HAM runs the PE at reduced clock until it has been busy
        # ~3.4us. A few dummy matmuls bridge engine-up (~8.4us) to first-data
        # (~10us); the promotion then completes on real rows.
        warm = wp.tile([128, TN], bf16)
        nc.vector.memset(warm[:], 0.0)
        wps = pspool.tile([128, TN], f32, tag="ps", name="warmps")
        for _ in range(WARM_N):
            nc.tensor.matmul(wps[:], warm[:, :128], warm[:], start=True, stop=True)

        # patch tiles back-to-back; cls groups ride inside tile 3's slots
        for t, xt in enumerate([xt0, xt1, xt2, xt3]):
            patch_tile(t, xt)

    nc.compile()
    _NC_CACHE[rt.sig] = nc
    return nc


# ---------------------------------------------------------------- host glue
def _bf(a):
    return np.ascontiguousarray(np.asarray(a), dtype=ml_dtypes.bfloat16)


def _f32(a):
    return np.ascontiguousarray(np.asarray(a), dtype=np.float32)


def _shard_inputs(rt, x, moe0_W1, moe0_b1, moe0_W2, moe0_b2, A1_W, A1_b, A2_W, A2_b):
    x = np.asarray(x, np.float32)

    # shared (replicated) tensors
    # [d, h] -> [p, hm, k, c] with d = k*128+p, h = hm*128+c
    w1v = _bf(np.asarray(moe0_W1, np.float32)).reshape(KD, 128, MH, 128)
    w1v = np.ascontiguousarray(w1v.transpose(1, 2, 0, 3))
    w2v = _bf(np.asarray(moe0_W2, np.float32)).reshape(MH, 128, D).transpose(1, 0, 2)
    w2v = np.ascontiguousarray(w2v)
    b1v = np.ascontiguousarray(_f32(moe0_b1).reshape(MH, 128).T)
    b2v = np.ascontiguousarray(_f32(moe0_b2).reshape(KD, 128).T)

    # stacked selected cls columns (L1 / atom1-group order) -> [128, KD, NSEL]
    xc_f = x[:, :NCLS, :]                                   # [B, 6, D]
    cols = [xc_f[b, i, :] for j in range(5)
            for (i, a, c, bs) in rt.bl1[j] for b in bs]
    stacked = np.stack(cols, axis=0)                        # [NSEL, D]
    xcv = _bf(stacked.T.reshape(KD, 128, rt.NSEL).transpose(1, 0, 2))

    A1_W = np.asarray(A1_W, np.float32)
    A2_W = np.asarray(A2_W, np.float32)
    A1_b = np.asarray(A1_b, np.float32)

    in_maps = []
    for core in range(NCORES):
        hs = slice(core * HS, (core + 1) * HS)
        # per-core patch tokens, tile-major: [128, NT, KD, TN]
        xpc = x[core * BPC:(core + 1) * BPC, NCLS:, :].reshape(TPC, D)
        xpv = _bf(xpc.reshape(NT, TN, KD, 128).transpose(3, 0, 2, 1))
        # atom slices
        a1v = _bf(A1_W[:, :, hs].reshape(5, KD, 128, HS).transpose(0, 2, 1, 3))
        a2v = _bf(A2_W[:, hs, :].reshape(5, HK, 128, D).transpose(0, 2, 1, 3))
        a1bv = np.ascontiguousarray(
            A1_b[:, hs].reshape(5, HK, 128).transpose(2, 0, 1))
        in_maps.append({
            "xp": xpv, "w1": w1v, "w2": w2v, "b1": b1v, "b2": b2v,
            "xc": xcv, "a1": a1v, "a1b": a1bv, "a2": a2v,
        })
    return in_maps


def _combine_outputs(rt, results, A2_b):
    A2_b = np.asarray(A2_b, np.float32)
    out = np.empty((B, NCLS + PT, D), np.float32)
    for core in range(NCORES):
        ypv = np.asarray(results[core]["yp"], np.float32)  # [128, KD, TPC] bf16
        out[core * BPC:(core + 1) * BPC, NCLS:, :] = (
            ypv.transpose(2, 1, 0).reshape(BPC, PT, D))

    ycs = np.zeros((128, rt.NSEL, KD), np.float64)
    for core in range(NCORES):
        ycs += results[core]["yc"].reshape(128, rt.NSEL, KD)
    # [128, col, KD] -> [col, D] with D = kd*128 + p
    cols = ycs.transpose(1, 2, 0).reshape(rt.NSEL, D).astype(np.float32)
    cls_out = np.zeros((B, NCLS, D), np.float32)   # tie rows stay exactly 0
    for col, (b, i, c) in enumerate(rt.colmap):
        cls_out[b, i] = cols[col] + A2_b[c]
    out[:, :NCLS, :] = cls_out
    return out


def _run(inputs, trace=False, trace_kwargs=None):
    from concourse.bass_utils import run_bass_kernel_spmd

    g = _gates(inputs["x"], inputs["G_W"])
    rt = _Routing(g)
    nc = _build_nc(rt)
    in_maps = _shard_inputs(
        rt, inputs["x"], inputs["moe0_W1"], inputs["moe0_b1"], inputs["moe0_W2"],
        inputs["moe0_b2"], inputs["A1_W"], inputs["A1_b"], inputs["A2_W"],
        inputs["A2_b"])
    res = run_bass_kernel_spmd(nc, in_maps, core_ids=list(range(NCORES)),
                               trace=trace, **(trace_kwargs or {}))
    out = _combine_outputs(rt, res.results, inputs["A2_b"])
    return out, res


def kernel(**inputs) -> np.ndarray:
    out, _ = _run(inputs, trace=False)
    return out
